# revision 2
# baseline (speedup 1.0000x reference)
"""Trainium2 Bass kernel for nn_HMM_80410377716208.

Math
----
reference computes, with q = softmax(q_logits), e = q @ sigmoid(emission_logits):
  rec_losses[b,t] = -sum_d [ x*log(e+EPS) + (1-x)*log(1-e+EPS) ]
                  = -( C0 + x[b,t,:] . w ),   w = log(e+EPS)-log(1-e+EPS),
                                              C0 = sum_d log(1-e+EPS)
  rec_loss = sum_{b, t<len_b} rec_losses / R,  R = sum(len_b)
  kl_loss  = (kl0 * n0 + klt * (R - n0)) / R,  n0 = #batches with len_b >= 1

The only large-data computation is the masked sum
  v[d] = sum_{b, t<len_b} x[b,t,d]
which is permutation-invariant over valid (b,t) rows.  x is exactly 0/1
(binary Bernoulli data), so v is integer-exact and the rows transport
losslessly in fp8e4m3 (4x less DMA traffic than f32).

Strategy (8 NeuronCores, data-parallel as per the sharding hint)
----------------------------------------------------------------
host:   gather valid rows, redistribute them evenly over the 8 cores
        (zero-padding to 128-row chunks; zero rows contribute nothing),
        cast 0/1 -> fp8.
device: per core, stream its [NC, 128, 512] chunk array through SBUF and
        accumulate ones^T @ X into one fp32 PSUM bank on the TensorEngine
        -> exact per-core column sums v_c [1, 512].
host:   v = sum_c v_c (the "all-reduce" of the hint, 8x512 floats), then
        the scalar epilogue above in float64.
"""

import sys

sys.path.insert(0, "/opt/trn_rl_repo")

import numpy as np

from concourse import bacc, mybir
from concourse.tile import TileContext
from concourse.bass_utils import run_bass_kernel_spmd

B, T, D, Z = 128, 512, 512, 64
EPS = 1e-10
N_CORES = 8
GROUP = 8          # 128-row chunks per DMA (512 KB per dma_start at fp8)
XPOOL_BUFS = 3

KDT = mybir.dt.float8e4          # on-device dtype for x / ones
NP_KDT = mybir.dt.np(KDT)
F32 = mybir.dt.float32

# bit pattern of 1.0 in the kernel dtype, for cheap 0/1 -> KDT packing
_ONE_BITS = np.ones((), NP_KDT).view(
    np.uint8 if np.dtype(NP_KDT).itemsize == 1 else np.uint16
)

TRACE = False          # set by test harness; collects perf info into LAST_PERF
LAST_PERF = {}

_cache = {}


def _build(nc_chunks: int):
    """Bass program: xp [NC,128,D] KDT, ones [128,1] KDT -> v [1,D] f32."""
    groups = [GROUP] * (nc_chunks // GROUP)
    if nc_chunks % GROUP:
        groups.append(nc_chunks % GROUP)

    nc = bacc.Bacc(None, target_bir_lowering=False)
    x_in = nc.declare_dram_parameter("xp", [nc_chunks, 128, D], KDT, isOutput=False)
    ones_in = nc.declare_dram_parameter("ones", [128, 1], KDT, isOutput=False)
    v_out = nc.declare_dram_parameter("v", [1, D], F32, isOutput=True)

    with TileContext(nc) as tc:
        with (
            tc.tile_pool(name="const", bufs=1) as cpool,
            tc.tile_pool(name="xb", bufs=XPOOL_BUFS) as xpool,
            tc.tile_pool(name="psum", bufs=1, space="PSUM") as ppool,
        ):
            ones_sb = cpool.tile([128, 1], KDT)
            nc.sync.dma_start(ones_sb[:], ones_in[:])
            # pre-touch ones on PE so the first real matmul carries only its
            # own x-DMA wait (Matmult HW allows a single sync wait)
            scratch = ppool.tile([1, 1], F32)
            nc.tensor.matmul(scratch[:], ones_sb[:], ones_sb[:, :1])

            acc = ppool.tile([1, D], F32)
            n_mm = sum(groups)
            mm = 0
            ofs = 0
            for g in groups:
                xt = xpool.tile([128, g, D], KDT)
                # one DMA per group: dram [g, 128, D] -> sbuf [128, g, D]
                nc.sync.dma_start(
                    xt[:], x_in[ofs : ofs + g].rearrange("g p d -> p g d")
                )
                for k in range(g):
                    nc.tensor.matmul(
                        acc[:], ones_sb[:], xt[:, k, :],
                        start=(mm == 0), stop=(mm == n_mm - 1),
                    )
                    mm += 1
                ofs += g
            acc_sb = cpool.tile([1, D], F32)
            nc.vector.tensor_copy(acc_sb[:], acc[:])
            nc.sync.dma_start(v_out[:], acc_sb[:])
    nc.compile()
    return nc


def _get_program(nc_chunks: int):
    if nc_chunks not in _cache:
        _cache[nc_chunks] = _build(nc_chunks)
    return _cache[nc_chunks]


def _pack_rows(x: np.ndarray, lens: np.ndarray, nc_chunks: int) -> np.ndarray:
    """Gather valid rows of x, 0/1 -> KDT, pad, shape [N_CORES, NC, 128, D]."""
    rows_total = N_CORES * nc_chunks * 128
    xa = x.reshape(B * T, D)
    starts = np.arange(B, dtype=np.int64) * T
    idx = np.concatenate(
        [starts[b] + np.arange(lens[b], dtype=np.int64) for b in range(B)]
    )
    buf = np.zeros((rows_total, D), dtype=_ONE_BITS.dtype)
    np.multiply(xa[idx] != 0, _ONE_BITS, out=buf[: len(idx)], casting="unsafe")
    return buf.view(NP_KDT).reshape(N_CORES, nc_chunks, 128, D)


def _softmax64(v):
    v = np.asarray(v, np.float64)
    m = v.max(axis=-1, keepdims=True)
    e = np.exp(v - m)
    return e / e.sum(axis=-1, keepdims=True)


def kernel(x, x_lens, transition_logits, emission_logits, initial_logits, q_logits):
    x = np.asarray(x)
    lens = np.clip(np.asarray(x_lens, np.int64), 0, T)
    R = int(lens.sum())
    n0 = int((lens >= 1).sum())

    # ---- tiny parameter math (host, f64) ----
    q = _softmax64(np.asarray(q_logits, np.float64))[0]          # [Z]
    p0 = _softmax64(np.asarray(initial_logits, np.float64))      # [Z]
    kl0 = float(np.sum(q * (np.log(q + EPS) - np.log(p0 + EPS))))
    A = _softmax64(np.asarray(transition_logits, np.float64))    # [Z, Z] rows
    p_next = q @ A
    p_next_probs = _softmax64(np.log(p_next + EPS))
    klt = float(np.sum(q * (np.log(q + EPS) - np.log(p_next_probs + EPS))))
    e = q @ (1.0 / (1.0 + np.exp(-np.asarray(emission_logits, np.float64))))  # [D]
    log_e = np.log(e + EPS)
    log_1me = np.log(1.0 - e + EPS)
    w = log_e - log_1me                                           # [D]
    C0 = float(np.sum(log_1me))

    if R == 0:
        nan = np.float32(np.nan)
        return (nan, nan)

    # ---- heavy masked column-sum on the 8 NeuronCores ----
    nc_chunks = -(-R // (N_CORES * 128))          # ceil
    packed = _pack_rows(x, lens, nc_chunks)
    ones = np.ones((128, 1), NP_KDT)
    nc = _get_program(nc_chunks)
    in_maps = [{"xp": packed[c], "ones": ones} for c in range(N_CORES)]
    res = run_bass_kernel_spmd(
        nc, in_maps, core_ids=list(range(N_CORES)), trace=TRACE
    )
    if TRACE:
        LAST_PERF.clear()
        LAST_PERF.update(
            exec_time_ns=res.exec_time_ns,
            mean_exec_time_ns=res.mean_exec_time_ns,
            max_exec_time_core_id=res.max_exec_time_core_id,
            trace=res.instructions_and_trace[1] if res.instructions_and_trace else None,
        )
    v = np.zeros(D, np.float64)
    for c in range(N_CORES):
        v += res.results[c]["v"][0].astype(np.float64)

    rec_loss = -(C0 * R + float(v @ w)) / R
    kl_loss = (kl0 * n0 + klt * (R - n0)) / R
    return (np.float32(rec_loss), np.float32(kl_loss))
